# revision 2
# baseline (speedup 1.0000x reference)
"""Correlation-layer (cost volume) kernel for 8 Trainium2 NeuronCores.

Problem: out[n, 0, h, w, dy*41+dx] = sum_c fm1[n,c,h,w] * fm2p[n,c,h+dy,w+dx]
with fm2p = fm2 zero-padded by 20 on both spatial axes, dy,dx in [0,41).

Sharding: core k handles batch n = k//2 and h-slab [64*(k%2), 64*(k%2)+64).
No cross-core communication (fm2 slab + halo prepared on the host).

Device algorithm (per core, fp16 in / fp32 PSUM / fp16 band out):
  - PE runs in 64x32 tiling mode: 8 independent tiles = (h parity s) x
    (w quadrant q).  Row tile s uses SBUF partitions [64s,64s+64) (fm1/fm2
    channel data duplicated per half on the host); col tile q owns PSUM
    partitions [32q,32q+32) = w in [32q,32q+32).
  - Stationary lhsT = fm1[c, w-quadrant] loaded once per (hp,s,q) via
    ldweights; 6 chunk matmuls reuse it (InstMatmult.ldweights=False).
  - Moving rhs = fm2t[c, col, row] (host-transposed so the innermost
    streamed dim is contiguous): chunk j streams cols [32q+12j,+12) x 41 dy
    -> PSUM [w, col(12), dy(41)] = 492 <= one 2KB bank.
  - DVE and ACT alternate evacuating chunks (fp32->fp16) into the band
    tile S[128, 2, 72, 41] = [w, s, col-32q, dy].
  - One 128-partition DMA per h-pair writes S to HBM at line rate
    (measured: narrow-partition DMAs collapse to ~1/4 bandwidth, and
    per-partition diagonal extraction is not expressible in DMA APs, so
    the 72/41 band inflation is kept and sheared on the host).
  - Host extracts dx = col - (w mod 32) per w with zero-copy as_strided
    views during the fp32 upcast.
"""

import os
import sys

import numpy as np

for _p in ("/opt/trn_rl_repo",):
    if os.path.isdir(_p) and _p not in sys.path:
        sys.path.append(_p)

# ---- problem constants (hardcoded per contest rules) ----
B, C, H, W = 4, 64, 128, 128
MD = 20                  # max displacement
D = 2 * MD + 1           # 41 displacements per axis
PW = W + 2 * MD          # 168 padded width
HS = H // 2              # 64-row h-slab per core
RS = HS + 2 * MD         # 104 fm2 slab rows (with halo)
NCORES = 8

MQ = 32                  # w-quadrant width (PE col-tile size)
NQ = W // MQ             # 4 col quadrants
WIN = MQ + 2 * MD        # 72-col band window per quadrant
CCH = 12                 # cols per PSUM chunk (12*41=492 <= 512 bank)
NCH = WIN // CCH         # 6 chunks

# DVE(0.96GHz):ACT(1.2GHz) weighted evac pattern, A=ACT, V=DVE
_EVAC_PATTERN = "AVAVAAVAV"

_CACHE = {}


def _build_program(io_dtype_name="float16", loop_k=0, ldw_reuse=True):
    """Build + compile the single-core SPMD Bass program.

    loop_k > 0 builds a TIMING variant: the compute loop runs loop_k times
    inside a device-side For_i, output goes to Internal DRAM, and only a tiny
    marker tensor is an ExternalOutput.
    """
    import contextlib

    from concourse import bacc
    import concourse.mybir as mybir
    import concourse.tile as tile

    dt_io = getattr(mybir.dt, io_dtype_name)

    nc = bacc.Bacc("TRN2", target_bir_lowering=False, debug=False)
    fm1_d = nc.dram_tensor("fm1s", [128, HS // 2, W], dt_io, kind="ExternalInput").ap()
    fm2_d = nc.dram_tensor("fm2t", [128, PW, RS], dt_io, kind="ExternalInput").ap()
    out_kind = "Internal" if loop_k else "ExternalOutput"
    out_d = nc.dram_tensor(
        "outs", [HS // 2, 128, 2, WIN, D], dt_io, kind=out_kind
    ).ap()
    marker_d = None
    if loop_k:
        marker_d = nc.dram_tensor(
            "marker", [1, 8], mybir.dt.float32, kind="ExternalOutput"
        ).ap()

    with tile.TileContext(nc) as tc:
        with (
            tc.tile_pool(name="const", bufs=1) as cpool,
            tc.tile_pool(name="srow", bufs=3) as spool,
            tc.tile_pool(name="psum", bufs=1, space="PSUM") as ppool,
        ):
            fm1_sb = cpool.tile([128, HS // 2, W], dt_io)
            fm2_sb = cpool.tile([128, PW, RS], dt_io)
            nc.sync.dma_start(fm1_sb[:], fm1_d[:])
            nc.sync.dma_start(fm2_sb[:], fm2_d[:])

            evac_i = 0
            loop_cm = tc.For_i(0, loop_k, 1) if loop_k else contextlib.nullcontext()
            with loop_cm:
                for hp in range(HS // 2):
                    S = spool.tile([128, 2, WIN, D], dt_io, tag="S")
                    for s in range(2):
                        r0 = 2 * hp + s
                        ps = [
                            ppool.tile(
                                [128, 512], mybir.dt.float32,
                                name=f"ps{j}", tag=f"ps{j}",
                            )
                            for j in range(NCH)
                        ]
                        for q in range(NQ):
                            lhsT = fm1_sb[
                                64 * s : 64 * s + 64, hp, MQ * q : MQ * (q + 1)
                            ]
                            if ldw_reuse:
                                nc.tensor.ldweights(
                                    lhsT, tile_position=(64 * s, MQ * q)
                                )
                            for j in range(NCH):
                                c0 = MQ * q + CCH * j
                                mm = nc.tensor.matmul(
                                    ps[j][MQ * q : MQ * (q + 1), 0 : CCH * D],
                                    lhsT,
                                    fm2_sb[
                                        64 * s : 64 * s + 64,
                                        c0 : c0 + CCH,
                                        r0 : r0 + D,
                                    ],
                                    start=True,
                                    stop=True,
                                    tile_position=(64 * s, MQ * q),
                                )
                                if ldw_reuse:
                                    mm.ldweights = False
                        for j in range(NCH):
                            use_act = _EVAC_PATTERN[evac_i % len(_EVAC_PATTERN)] == "A"
                            evac_i += 1
                            copy = nc.scalar.copy if use_act else nc.vector.tensor_copy
                            copy(
                                S[:, s, CCH * j : CCH * (j + 1), :],
                                ps[j][:, 0 : CCH * D],
                            )
                    nc.sync.dma_start(out_d[hp], S[:])

            if loop_k:
                mk = cpool.tile([1, 8], mybir.dt.float32, name="mk")
                nc.vector.memset(mk[:], 1.0)
                nc.sync.dma_start(marker_d[:], mk[:])

    nc.compile()
    return nc


def _get_compiled(io_dtype_name="float16", loop_k=0, ldw_reuse=True):
    key = ("prog", io_dtype_name, loop_k, ldw_reuse)
    if key not in _CACHE:
        _CACHE[key] = _build_program(io_dtype_name, loop_k, ldw_reuse)
    return _CACHE[key]


def shard_inputs(fm1, fm2, np_dtype=np.float16):
    """Full (4,64,128,128) inputs -> 8 per-core input dicts."""
    fm1 = np.asarray(fm1, dtype=np.float32)
    fm2 = np.asarray(fm2, dtype=np.float32)
    in_maps = []
    for k in range(NCORES):
        n, hbase = k // 2, (k % 2) * HS
        a = fm1[n].astype(np_dtype)                      # (C, H, W)
        slab = a[:, hbase : hbase + HS]                  # (C, 64, W)
        fm1s = np.concatenate([slab[:, 0::2], slab[:, 1::2]], axis=0)
        fm1s = np.ascontiguousarray(fm1s)                # (128, 32, W)

        p = np.zeros((C, H + 2 * MD, PW), dtype=np_dtype)
        p[:, MD : MD + H, MD : MD + W] = fm2[n].astype(np_dtype)
        slab2 = p[:, hbase : hbase + RS]                 # (C, 104, 168)
        slab2t = slab2.transpose(0, 2, 1)                # (C, 168, 104)
        fm2t = np.ascontiguousarray(np.concatenate([slab2t, slab2t], axis=0))
        in_maps.append({"fm1s": fm1s, "fm2t": fm2t})
    return in_maps


def unshard_outputs(results):
    """8 per-core {'outs': (32,128,2,72,41)} -> full (4,1,128,128,1681) fp32."""
    out = np.empty((B, 1, H, W, D * D), dtype=np.float32)
    for k in range(NCORES):
        n, hbase = k // 2, (k % 2) * HS
        g = np.asarray(results[k]["outs"])               # [hp, p, s, col, dy]
        st = g.strides
        dst = out[n, 0, hbase : hbase + HS].reshape(HS // 2, 2, W, D * D)
        for q in range(NQ):
            vq = np.lib.stride_tricks.as_strided(
                g[:, MQ * q :],
                shape=(HS // 2, 2, MQ, D, D),
                strides=(st[0], st[2], st[1] + st[3], st[4], st[3]),
            )  # [hp, s, m, dy, dx]
            dst[:, :, MQ * q : MQ * (q + 1), :] = (
                vq.astype(np.float32).reshape(HS // 2, 2, MQ, D * D)
            )
    return out


def run_on_hw(in_maps, io_dtype_name="float16", trace=False, **kw):
    from concourse import bass_utils

    nc = _get_compiled(io_dtype_name)
    res = bass_utils.run_bass_kernel_spmd(
        nc, in_maps, list(range(NCORES)), trace=trace, **kw
    )
    return res


def kernel(feature_map_1, feature_map_2):
    in_maps = shard_inputs(feature_map_1, feature_map_2)
    res = run_on_hw(in_maps)
    return unshard_outputs(res.results)


if __name__ == "__main__":
    inputs = {
        "feature_map_1": np.random.randn(B, C, H, W).astype(np.float32),
        "feature_map_2": np.random.randn(B, C, H, W).astype(np.float32),
    }
    out = kernel(**inputs)
    print("kernel output", out.shape, out.dtype)


# revision 3
# speedup vs baseline: 1.7475x; 1.7475x over previous
"""Correlation-layer (cost volume) kernel for 8 Trainium2 NeuronCores.

Problem: out[n, 0, h, w, dy*41+dx] = sum_c fm1[n,c,h,w] * fm2p[n,c,h+dy,w+dx]
with fm2p = fm2 zero-padded by 20 on both spatial axes, dy,dx in [0,41).

Sharding: core k handles batch n = k//2 and h-slab [64*(k%2), 64*(k%2)+64).
No cross-core communication (fm2 slab + halo prepared on the host).

Device algorithm (per core, fp16 in / fp32 PSUM / fp16 band out):
  - PE runs in 64x32 tiling mode: 8 independent tiles = (h parity s) x
    (w quadrant q).  Row tile s uses SBUF partitions [64s,64s+64) (fm1/fm2
    channel data duplicated per half on the host); col tile q owns PSUM
    partitions [32q,32q+32) = w in [32q,32q+32).
  - Stationary lhsT = fm1[c, w-quadrant] loaded once per (hp,s,q) via
    ldweights; 6 chunk matmuls reuse it (InstMatmult.ldweights=False).
  - Moving rhs = fm2t[c, col, row] (host-transposed so the innermost
    streamed dim is contiguous): chunk j streams cols [32q+12j,+12) x 41 dy
    -> PSUM [w, col(12), dy(41)] = 492 <= one 2KB bank.
  - DVE and ACT alternate evacuating chunks (fp32->fp16) into the band
    tile S[128, 2, 72, 41] = [w, s, col-32q, dy].
  - One 128-partition DMA per h-pair writes S to HBM at line rate
    (measured: narrow-partition DMAs collapse to ~1/4 bandwidth, and
    per-partition diagonal extraction is not expressible in DMA APs, so
    the 72/41 band inflation is kept and sheared on the host).
  - Host extracts dx = col - (w mod 32) per w with zero-copy as_strided
    views during the fp32 upcast.
"""

import os
import sys

import numpy as np

for _p in ("/opt/trn_rl_repo",):
    if os.path.isdir(_p) and _p not in sys.path:
        sys.path.append(_p)

# ---- problem constants (hardcoded per contest rules) ----
B, C, H, W = 4, 64, 128, 128
MD = 20                  # max displacement
D = 2 * MD + 1           # 41 displacements per axis
PW = W + 2 * MD          # 168 padded width
HS = H // 2              # 64-row h-slab per core
RS = HS + 2 * MD         # 104 fm2 slab rows (with halo)
NCORES = 8

MQ = 32                  # w-quadrant width (PE col-tile size)
NQ = W // MQ             # 4 col quadrants
WIN = MQ + 2 * MD        # 72-col band window per quadrant
CCH = 12                 # cols per PSUM chunk (12*41=492 <= 512 bank)
NCH = WIN // CCH         # 6 chunks

# DVE(0.96GHz):ACT(1.2GHz) weighted evac pattern, A=ACT, V=DVE
_EVAC_PATTERN = "AVAVAAVAV"

_CACHE = {}


def _build_program(io_dtype_name="float16", loop_k=0, ldw_reuse=True):
    """Build + compile the single-core SPMD Bass program.

    loop_k > 0 builds a TIMING variant: the compute loop runs loop_k times
    inside a device-side For_i, output goes to Internal DRAM, and only a tiny
    marker tensor is an ExternalOutput.
    """
    import contextlib

    from concourse import bacc
    import concourse.mybir as mybir
    import concourse.tile as tile

    dt_io = getattr(mybir.dt, io_dtype_name)

    nc = bacc.Bacc("TRN2", target_bir_lowering=False, debug=False)
    fm1_d = nc.dram_tensor("fm1s", [128, HS // 2, W], dt_io, kind="ExternalInput").ap()
    fm2_d = nc.dram_tensor("fm2t", [128, PW, RS], dt_io, kind="ExternalInput").ap()
    out_kind = "Internal" if loop_k else "ExternalOutput"
    out_d = nc.dram_tensor(
        "outs", [HS // 2, 128, 2, WIN, D], dt_io, kind=out_kind
    ).ap()
    marker_d = None
    if loop_k:
        marker_d = nc.dram_tensor(
            "marker", [1, 8], mybir.dt.float32, kind="ExternalOutput"
        ).ap()

    with tile.TileContext(nc) as tc:
        with (
            tc.tile_pool(name="const", bufs=1) as cpool,
            tc.tile_pool(name="srow", bufs=4) as spool,
            tc.tile_pool(name="psum", bufs=3, space="PSUM") as ppool,
        ):
            fm1_sb = cpool.tile([128, HS // 2, W], dt_io)
            fm2_sb = cpool.tile([128, PW, RS], dt_io)
            nc.sync.dma_start(fm1_sb[:], fm1_d[:])
            # split fm2t by col range so early matmuls start sooner
            # (tile subtile deps gate each chunk on its own piece)
            for c0, c1 in ((0, 56), (56, 112), (112, PW)):
                nc.sync.dma_start(fm2_sb[:, c0:c1, :], fm2_d[:, c0:c1, :])

            evac_i = 0
            loop_cm = tc.For_i(0, loop_k, 1) if loop_k else contextlib.nullcontext()
            with loop_cm:
                for hp in range(HS // 2):
                    S = spool.tile([128, 2, WIN, D], dt_io, tag="S")
                    for s in range(2):
                        r0 = 2 * hp + s
                        for j in range(NCH):
                            ps = ppool.tile(
                                [128, 512], mybir.dt.float32,
                                name=f"ps{s}", tag=f"ps{s}",
                            )
                            for q in range(NQ):
                                c0 = MQ * q + CCH * j
                                nc.tensor.matmul(
                                    ps[MQ * q : MQ * (q + 1), 0 : CCH * D],
                                    fm1_sb[
                                        64 * s : 64 * s + 64, hp,
                                        MQ * q : MQ * (q + 1),
                                    ],
                                    fm2_sb[
                                        64 * s : 64 * s + 64,
                                        c0 : c0 + CCH,
                                        r0 : r0 + D,
                                    ],
                                    start=True,
                                    stop=True,
                                    tile_position=(64 * s, MQ * q),
                                )
                            use_act = _EVAC_PATTERN[evac_i % len(_EVAC_PATTERN)] == "A"
                            evac_i += 1
                            copy = nc.scalar.copy if use_act else nc.vector.tensor_copy
                            copy(
                                S[:, s, CCH * j : CCH * (j + 1), :],
                                ps[:, 0 : CCH * D],
                            )
                    nc.sync.dma_start(out_d[hp], S[:])

            if loop_k:
                mk = cpool.tile([1, 8], mybir.dt.float32, name="mk")
                nc.vector.memset(mk[:], 1.0)
                nc.sync.dma_start(marker_d[:], mk[:])

    nc.compile()
    return nc


def _get_compiled(io_dtype_name="float16", loop_k=0, ldw_reuse=True):
    key = ("prog", io_dtype_name, loop_k, ldw_reuse)
    if key not in _CACHE:
        _CACHE[key] = _build_program(io_dtype_name, loop_k, ldw_reuse)
    return _CACHE[key]


def shard_inputs(fm1, fm2, np_dtype=np.float16):
    """Full (4,64,128,128) inputs -> 8 per-core input dicts."""
    fm1 = np.asarray(fm1, dtype=np.float32)
    fm2 = np.asarray(fm2, dtype=np.float32)
    in_maps = []
    for k in range(NCORES):
        n, hbase = k // 2, (k % 2) * HS
        a = fm1[n].astype(np_dtype)                      # (C, H, W)
        slab = a[:, hbase : hbase + HS]                  # (C, 64, W)
        fm1s = np.concatenate([slab[:, 0::2], slab[:, 1::2]], axis=0)
        fm1s = np.ascontiguousarray(fm1s)                # (128, 32, W)

        p = np.zeros((C, H + 2 * MD, PW), dtype=np_dtype)
        p[:, MD : MD + H, MD : MD + W] = fm2[n].astype(np_dtype)
        slab2 = p[:, hbase : hbase + RS]                 # (C, 104, 168)
        slab2t = slab2.transpose(0, 2, 1)                # (C, 168, 104)
        fm2t = np.ascontiguousarray(np.concatenate([slab2t, slab2t], axis=0))
        in_maps.append({"fm1s": fm1s, "fm2t": fm2t})
    return in_maps


def unshard_outputs(results):
    """8 per-core {'outs': (32,128,2,72,41)} -> full (4,1,128,128,1681) fp32."""
    out = np.empty((B, 1, H, W, D * D), dtype=np.float32)
    for k in range(NCORES):
        n, hbase = k // 2, (k % 2) * HS
        g = np.asarray(results[k]["outs"])               # [hp, p, s, col, dy]
        st = g.strides
        dst = out[n, 0, hbase : hbase + HS].reshape(HS // 2, 2, W, D * D)
        for q in range(NQ):
            vq = np.lib.stride_tricks.as_strided(
                g[:, MQ * q :],
                shape=(HS // 2, 2, MQ, D, D),
                strides=(st[0], st[2], st[1] + st[3], st[4], st[3]),
            )  # [hp, s, m, dy, dx]
            dst[:, :, MQ * q : MQ * (q + 1), :] = (
                vq.astype(np.float32).reshape(HS // 2, 2, MQ, D * D)
            )
    return out


def run_on_hw(in_maps, io_dtype_name="float16", trace=False, **kw):
    from concourse import bass_utils

    nc = _get_compiled(io_dtype_name)
    res = bass_utils.run_bass_kernel_spmd(
        nc, in_maps, list(range(NCORES)), trace=trace, **kw
    )
    return res


def kernel(feature_map_1, feature_map_2):
    in_maps = shard_inputs(feature_map_1, feature_map_2)
    res = run_on_hw(in_maps)
    return unshard_outputs(res.results)


if __name__ == "__main__":
    inputs = {
        "feature_map_1": np.random.randn(B, C, H, W).astype(np.float32),
        "feature_map_2": np.random.randn(B, C, H, W).astype(np.float32),
    }
    out = kernel(**inputs)
    print("kernel output", out.shape, out.dtype)


# revision 5
# speedup vs baseline: 2.0033x; 1.1464x over previous
"""Correlation-layer (cost volume) kernel for 8 Trainium2 NeuronCores.

Problem: out[n, 0, h, w, dy*41+dx] = sum_c fm1[n,c,h,w] * fm2p[n,c,h+dy,w+dx]
with fm2p = fm2 zero-padded by 20 on both spatial axes, dy,dx in [0,41).

Sharding: core k handles batch n = k//2 and h-slab [64*(k%2), 64*(k%2)+64).
No cross-core communication (fm2 slab + halo prepared on the host).

Device algorithm (per core, fp16 in / fp32 PSUM / fp16 band out):
  - PE runs in 64x32 tiling mode: 8 independent tiles = (h parity s) x
    (w quadrant q).  Row tile s uses SBUF partitions [64s,64s+64) (fm1/fm2
    channel data duplicated per half on the host); col tile q owns PSUM
    partitions [32q,32q+32) = w in [32q,32q+32).
  - Stationary lhsT = fm1[c, w-quadrant] loaded once per (hp,s,q) via
    ldweights; 6 chunk matmuls reuse it (InstMatmult.ldweights=False).
  - Moving rhs = fm2t[c, col, row] (host-transposed so the innermost
    streamed dim is contiguous): chunk j streams cols [32q+12j,+12) x 41 dy
    -> PSUM [w, col(12), dy(41)] = 492 <= one 2KB bank.
  - DVE and ACT alternate evacuating chunks (fp32->fp16) into the band
    tile S[128, 2, 72, 41] = [w, s, col-32q, dy].
  - One 128-partition DMA per h-pair writes S to HBM at line rate
    (measured: narrow-partition DMAs collapse to ~1/4 bandwidth, and
    per-partition diagonal extraction is not expressible in DMA APs, so
    the 72/41 band inflation is kept and sheared on the host).
  - Host extracts dx = col - (w mod 32) per w with zero-copy as_strided
    views during the fp32 upcast.
"""

import os
import sys

import numpy as np

for _p in ("/opt/trn_rl_repo",):
    if os.path.isdir(_p) and _p not in sys.path:
        sys.path.append(_p)

# ---- problem constants (hardcoded per contest rules) ----
B, C, H, W = 4, 64, 128, 128
MD = 20                  # max displacement
D = 2 * MD + 1           # 41 displacements per axis
PW = W + 2 * MD          # 168 padded width
HS = H // 2              # 64-row h-slab per core
RS = HS + 2 * MD         # 104 fm2 slab rows (with halo)
NCORES = 8

MQ = 32                  # w-quadrant width (PE col-tile size)
NQ = W // MQ             # 4 col quadrants
WIN = MQ + 2 * MD        # 72-col band window per quadrant
CCH = 12                 # cols per PSUM chunk (12*41=492 <= 512 bank)
NCH = WIN // CCH         # 6 chunks

# DVE(0.96GHz):ACT(1.2GHz) weighted evac pattern, A=ACT, V=DVE
_EVAC_PATTERN = "AVAVAAVAV"

_CACHE = {}


def _build_program(io_dtype_name="float16", loop_k=0, ldw_reuse=True):
    """Build + compile the single-core SPMD Bass program.

    loop_k > 0 builds a TIMING variant: the compute loop runs loop_k times
    inside a device-side For_i, output goes to Internal DRAM, and only a tiny
    marker tensor is an ExternalOutput.
    """
    import contextlib

    from concourse import bacc
    import concourse.mybir as mybir
    import concourse.tile as tile

    dt_io = getattr(mybir.dt, io_dtype_name)

    nc = bacc.Bacc("TRN2", target_bir_lowering=False, debug=False)
    fm1_d = nc.dram_tensor("fm1s", [128, HS // 2, W], dt_io, kind="ExternalInput").ap()
    fm2_d = nc.dram_tensor("fm2t", [128, PW, RS], dt_io, kind="ExternalInput").ap()
    out_kind = "Internal" if loop_k else "ExternalOutput"
    out_d = nc.dram_tensor(
        "outs", [HS // 2, 128, 2, WIN, D], dt_io, kind=out_kind
    ).ap()
    marker_d = None
    if loop_k:
        marker_d = nc.dram_tensor(
            "marker", [1, 8], mybir.dt.float32, kind="ExternalOutput"
        ).ap()

    with tile.TileContext(nc) as tc:
        with (
            tc.tile_pool(name="const", bufs=1) as cpool,
            tc.tile_pool(name="srow", bufs=4) as spool,
            tc.tile_pool(name="psum", bufs=4, space="PSUM") as ppool,
        ):
            fm1_sb = cpool.tile([128, HS // 2, W], dt_io)
            fm2_sb = cpool.tile([128, PW, RS], dt_io)
            nc.sync.dma_start(fm1_sb[:], fm1_d[:])
            # split fm2t by col range so early matmuls start sooner
            # (tile subtile deps gate each chunk on its own piece)
            for c0, c1 in ((0, 56), (56, 112), (112, PW)):
                nc.sync.dma_start(fm2_sb[:, c0:c1, :], fm2_d[:, c0:c1, :])

            evac_i = 0
            loop_cm = tc.For_i(0, loop_k, 1) if loop_k else contextlib.nullcontext()
            with loop_cm:
                for hp in range(HS // 2):
                    S = spool.tile([128, 2, WIN, D], dt_io, tag="S")
                    for j in range(NCH):
                        for s in range(2):
                            r0 = 2 * hp + s
                            ps = ppool.tile(
                                [128, 512], mybir.dt.float32,
                                name=f"ps{s}", tag=f"ps{s}",
                            )
                            for q in range(NQ):
                                c0 = MQ * q + CCH * j
                                nc.tensor.matmul(
                                    ps[MQ * q : MQ * (q + 1), 0 : CCH * D],
                                    fm1_sb[
                                        64 * s : 64 * s + 64, hp,
                                        MQ * q : MQ * (q + 1),
                                    ],
                                    fm2_sb[
                                        64 * s : 64 * s + 64,
                                        c0 : c0 + CCH,
                                        r0 : r0 + D,
                                    ],
                                    start=True,
                                    stop=True,
                                    tile_position=(64 * s, MQ * q),
                                )
                            copy = (
                                nc.scalar.copy if evac_i % 2 == 0
                                else nc.vector.tensor_copy
                            )
                            evac_i += 1
                            copy(
                                S[:, s, CCH * j : CCH * (j + 1), :],
                                ps[:, 0 : CCH * D],
                            )
                    for s in range(2):
                        nc.sync.dma_start(out_d[hp, :, s], S[:, s])

            if loop_k:
                mk = cpool.tile([1, 8], mybir.dt.float32, name="mk")
                nc.vector.memset(mk[:], 1.0)
                nc.sync.dma_start(marker_d[:], mk[:])

    nc.compile()
    return nc


def _get_compiled(io_dtype_name="float16", loop_k=0, ldw_reuse=True):
    key = ("prog", io_dtype_name, loop_k, ldw_reuse)
    if key not in _CACHE:
        _CACHE[key] = _build_program(io_dtype_name, loop_k, ldw_reuse)
    return _CACHE[key]


def shard_inputs(fm1, fm2, np_dtype=np.float16):
    """Full (4,64,128,128) inputs -> 8 per-core input dicts."""
    fm1 = np.asarray(fm1, dtype=np.float32)
    fm2 = np.asarray(fm2, dtype=np.float32)
    in_maps = []
    for k in range(NCORES):
        n, hbase = k // 2, (k % 2) * HS
        a = fm1[n].astype(np_dtype)                      # (C, H, W)
        slab = a[:, hbase : hbase + HS]                  # (C, 64, W)
        fm1s = np.concatenate([slab[:, 0::2], slab[:, 1::2]], axis=0)
        fm1s = np.ascontiguousarray(fm1s)                # (128, 32, W)

        p = np.zeros((C, H + 2 * MD, PW), dtype=np_dtype)
        p[:, MD : MD + H, MD : MD + W] = fm2[n].astype(np_dtype)
        slab2 = p[:, hbase : hbase + RS]                 # (C, 104, 168)
        slab2t = slab2.transpose(0, 2, 1)                # (C, 168, 104)
        fm2t = np.ascontiguousarray(np.concatenate([slab2t, slab2t], axis=0))
        in_maps.append({"fm1s": fm1s, "fm2t": fm2t})
    return in_maps


def unshard_outputs(results):
    """8 per-core {'outs': (32,128,2,72,41)} -> full (4,1,128,128,1681) fp32."""
    out = np.empty((B, 1, H, W, D * D), dtype=np.float32)
    for k in range(NCORES):
        n, hbase = k // 2, (k % 2) * HS
        g = np.asarray(results[k]["outs"])               # [hp, p, s, col, dy]
        st = g.strides
        dst = out[n, 0, hbase : hbase + HS].reshape(HS // 2, 2, W, D * D)
        for q in range(NQ):
            vq = np.lib.stride_tricks.as_strided(
                g[:, MQ * q :],
                shape=(HS // 2, 2, MQ, D, D),
                strides=(st[0], st[2], st[1] + st[3], st[4], st[3]),
            )  # [hp, s, m, dy, dx]
            dst[:, :, MQ * q : MQ * (q + 1), :] = (
                vq.astype(np.float32).reshape(HS // 2, 2, MQ, D * D)
            )
    return out


def run_on_hw(in_maps, io_dtype_name="float16", trace=False, **kw):
    from concourse import bass_utils

    nc = _get_compiled(io_dtype_name)
    res = bass_utils.run_bass_kernel_spmd(
        nc, in_maps, list(range(NCORES)), trace=trace, **kw
    )
    return res


def kernel(feature_map_1, feature_map_2):
    in_maps = shard_inputs(feature_map_1, feature_map_2)
    res = run_on_hw(in_maps)
    return unshard_outputs(res.results)


if __name__ == "__main__":
    inputs = {
        "feature_map_1": np.random.randn(B, C, H, W).astype(np.float32),
        "feature_map_2": np.random.randn(B, C, H, W).astype(np.float32),
    }
    out = kernel(**inputs)
    print("kernel output", out.shape, out.dtype)
